# revision 29
# baseline (speedup 1.0000x reference)
"""Multi-head attention (B=2, S=4096, D=512, H=8) on 8 TRN2 NeuronCores.

Sharding: core = (batch, head-pair). Each core projects q/k/v onto its two
heads' 128 dims over the full 4096-token sequence (no redundant work), runs
flash-style attention, and computes a partial output projection over its 128
cat dims. The host sums the 4 partial outputs per batch and adds bo.

exp() is split between the scalar engine (exact LUT) and the vector engine
(calibrated fp16 Schraudolph int-trick) to balance the two bottleneck
engines; OFFLOAD/16 of the score tiles take the vector path.
"""
import os
import sys

for _p in ("/opt/trn_rl_repo",):
    if _p not in sys.path:
        sys.path.insert(0, _p)

import numpy as np
from contextlib import ExitStack

import concourse.bass as bass
import concourse.bacc as bacc
import concourse.tile as tile
from concourse import mybir
from concourse.bass_utils import run_bass_kernel_spmd

F16 = mybir.dt.float16
F32 = mybir.dt.float32
I16 = mybir.dt.int16

D = 512          # d_model
DK = 64          # head dim
S = 4096         # sequence length
NCORES = 8
NSW = 8          # sweeps of 512 queries
NCH = 32         # 128-key chunks
NGRP = 8         # kv 512-key projection groups
NU = NSW * 32    # units: sweep x (16 granules x 2 heads)

# exp offload: units with u % 16 < OFFLOAD take the DVE fast-exp path
OFFLOAD = 7
LOG2E = 1.4426950408889634
EXP_A = float(0.125 * LOG2E * 1024.0)
EXP_B = float(15.0 * 1024.0 - 60.0)

LAST_RESULTS = None


def _build_kernel():
    nc = bacc.Bacc("TRN2", target_bir_lowering=False, debug=False,
                   num_devices=NCORES)

    qT = nc.dram_tensor("qT", [D, S], F16, kind="ExternalInput").ap()
    kT = nc.dram_tensor("kT", [D, S], F16, kind="ExternalInput").ap()
    vT = nc.dram_tensor("vT", [D, S], F16, kind="ExternalInput").ap()
    wq = nc.dram_tensor("wq", [D, 128], F16, kind="ExternalInput").ap()
    wk = nc.dram_tensor("wk", [D, 128], F16, kind="ExternalInput").ap()
    wv = nc.dram_tensor("wv", [D, 128], F16, kind="ExternalInput").ap()
    wo = nc.dram_tensor("wo", [128, D], F16, kind="ExternalInput").ap()
    bq = nc.dram_tensor("bq", [128], F32, kind="ExternalInput").ap()
    bk = nc.dram_tensor("bk", [128], F32, kind="ExternalInput").ap()
    bv = nc.dram_tensor("bv", [128], F32, kind="ExternalInput").ap()
    out = nc.dram_tensor("out", [S, D], F16, kind="ExternalOutput").ap()

    with tile.TileContext(nc) as tc:
        _emit(tc, qT, kT, vT, wq, wk, wv, wo, bq, bk, bv, out)

    nc.compile()
    return nc


def _emit(tc, qT, kT, vT, wq, wk, wv, wo, bq, bk, bv, out):
    nc = tc.nc
    Exp = mybir.ActivationFunctionType.Exp

    with ExitStack() as ctx:
        const = ctx.enter_context(tc.tile_pool(name="const", bufs=1))
        kvin = ctx.enter_context(tc.tile_pool(name="kvin", bufs=3))
        qinp = ctx.enter_context(tc.tile_pool(name="qinp", bufs=2))
        qhp = ctx.enter_context(tc.tile_pool(name="qhp", bufs=2))
        catp = ctx.enter_context(tc.tile_pool(name="catp", bufs=2))
        ptp = ctx.enter_context(tc.tile_pool(name="ptp", bufs=7))
        normp = ctx.enter_context(tc.tile_pool(name="normp", bufs=2))
        obp = ctx.enter_context(tc.tile_pool(name="obp", bufs=2))
        # PSUM: 3 x [128,1024] score tiles (6 banks) + 2 pv accumulators.
        # Projection tiles borrow "st" slots.
        stp = ctx.enter_context(tc.tile_pool(name="stp", bufs=3, space="PSUM"))
        pvpp = ctx.enter_context(tc.tile_pool(name="pvpp", bufs=2, space="PSUM"))
        pjp = stp

        # ---- persistent SBUF tensors -------------------------------------
        wq_sb = const.tile([128, 4 * 128], F16)   # [dm%128, m*128 + d2h]
        wk_sb = const.tile([128, 4 * 128], F16)
        wv_sb = const.tile([128, 4 * 128], F16)
        wo_sb = const.tile([128, D], F16)         # [cat dim, e]
        khT = const.tile([128, S], F16)           # [d2h, keys]
        vh = const.tile([128, NCH * 130], F16)    # per chunk: j*65+dd, col 64=1
        bqk = const.tile([128, 2], F32)           # col0 bq, col1 bk
        bv_sb = const.tile([1, 128], F32)
        bv_rep = const.tile([128, 128], F32)
        ones32 = const.tile([1, 128], F32)

        # ---- startup loads (critical-path first: wk -> kv group 0) -------
        nc.sync.dma_start(wk_sb[:].rearrange("p (m d) -> p m d", d=128),
                          wk.rearrange("(m p) d -> p m d", p=128))
        nc.gpsimd.dma_start(wq_sb[:].rearrange("p (m d) -> p m d", d=128),
                            wq.rearrange("(m p) d -> p m d", p=128))
        nc.vector.memset(ones32[:], 1.0)
        # ones column of every vh chunk (col 64 of each head block)
        vh_ones = vh[:, :].rearrange("p (a c) -> p a c", c=65)[:, :, 64:65]
        nc.vector.memset(vh_ones, 1.0)
        # preload the exp table set while startup DMAs run
        warm = normp.tile([1, 128], F16, tag="warm")
        nc.scalar.activation(warm[0:1, :], ones32[0:1, :], Exp)

        # ---- DMA: 512-key kv group / 512-query group (one start each) ----
        kv_tiles = {}

        def prefetch_kv(G, eng, split=None):
            kin = kvin.tile([128, 2048], F16, tag="kin", name="kin")
            vin = kvin.tile([128, 2048], F16, tag="vin", name="vin")
            kv_tiles[G] = (kin, vin)
            src_k = kT.rearrange("(m p) k -> p m k", p=128)[:, :, G * 512:(G + 1) * 512]
            src_v = vT.rearrange("(m p) k -> p m k", p=128)[:, :, G * 512:(G + 1) * 512]
            dst_k = kin[:].rearrange("p (m k) -> p m k", k=512)
            dst_v = vin[:].rearrange("p (m k) -> p m k", k=512)
            if split is None:
                split = nc.gpsimd if eng is nc.sync else nc.sync
            eng.dma_start(dst_k[:, 0:2], src_k[:, 0:2])
            split.dma_start(dst_k[:, 2:4], src_k[:, 2:4])
            eng.dma_start(dst_v[:, 0:2], src_v[:, 0:2])
            split.dma_start(dst_v[:, 2:4], src_v[:, 2:4])

        qin_tiles = {}

        def prefetch_q(s, eng):
            qin = qinp.tile([128, 2048], F16, tag="qin", name="qin")
            qin_tiles[s] = qin
            src = qT.rearrange("(m p) k -> p m k", p=128)[:, :, s * 512:(s + 1) * 512]
            eng.dma_start(qin[:].rearrange("p (m k) -> p m k", k=512), src)

        # ---- projections --------------------------------------------------
        qh_tiles = {}

        def emit_qproj(s):
            qin = qin_tiles.pop(s)
            qh = qhp.tile([128, 512], F16, tag="qh", name="qh")
            qh_tiles[s] = qh
            pj = pjp.tile([128, 512], F32, tag="st", name="pj")
            for m in range(4):
                nc.tensor.matmul(
                    pj[:],
                    wq_sb[:, m * 128:(m + 1) * 128],
                    qin[:, m * 512:(m + 1) * 512],
                    start=(m == 0), stop=(m == 3))
            nc.scalar.add(qh[:], pj[:], bqk[:, 0:1])

        def emit_kproj(G):
            kin, _ = kv_tiles[G]
            pj = pjp.tile([128, 512], F32, tag="st", name="pj")
            for m in range(4):
                nc.tensor.matmul(
                    pj[:],
                    wk_sb[:, m * 128:(m + 1) * 128],
                    kin[:, m * 512:(m + 1) * 512],
                    start=(m == 0), stop=(m == 3))
            nc.scalar.add(khT[:, G * 512:(G + 1) * 512], pj[:], bqk[:, 1:2])

        def emit_vproj(G, ci):
            _, vin = kv_tiles[G]
            cc = 4 * G + ci
            pj = pjp.tile([128, 128], F32, tag="st", name="pj")
            for m in range(4):
                nc.tensor.matmul(
                    pj[:],
                    vin[:, m * 512 + ci * 128: m * 512 + ci * 128 + 128],
                    wv_sb[:, m * 128:(m + 1) * 128],
                    start=(m == 0), stop=(m == 3))
            dst = vh[:, cc * 130:(cc + 1) * 130]
            dst = dst.rearrange("p (h c) -> p h c", c=65)[:, :, 0:64]
            nc.vector.tensor_add(
                dst,
                pj[:].rearrange("p (h c) -> p h c", c=64),
                bv_rep[:].rearrange("p (h c) -> p h c", c=64))

        # ---- output projection (partial: 128 cat dims) --------------------
        cat_tiles = {}
        ob_tiles = {}

        def emit_oproj(s, i):
            cat = cat_tiles[s]
            if i == 0:
                ob_tiles[s] = obp.tile([128, 2048], F16, tag="ob", name="ob")
            ob = ob_tiles[s]
            pj = pjp.tile([128, 512], F32, tag="st", name="pj")
            nc.tensor.matmul(pj[:], cat[:, i * 128:(i + 1) * 128], wo_sb[:])
            nc.scalar.copy(ob[:, i * 512:(i + 1) * 512], pj[:])
            if s == NSW - 1:
                # stream the tail out block by block
                nc.sync.dma_start(out[s * 512 + i * 128:s * 512 + (i + 1) * 128, :],
                                  ob[:, i * 512:(i + 1) * 512])
                if i == 3:
                    cat_tiles.pop(s)
                    ob_tiles.pop(s)
            elif i == 3:
                cat_tiles.pop(s)
                ob = ob_tiles.pop(s)
                dst = out[s * 512:(s + 1) * 512, :].rearrange(
                    "(i p) e -> p i e", p=128)
                nc.sync.dma_start(
                    dst, ob[:].rearrange("p (i e) -> p i e", e=512))

        # ---- attention pipeline -------------------------------------------
        # unit u: sweep s = u//32, g = (u%32)//2, head j = u%2
        # stages: SC at u, exp at u-2, PV at u-4
        pvp_tiles = {}
        st_tiles = {}
        pt_tiles = {}

        def unit(u):
            s, r = divmod(u, 32)
            return s, r // 2, r % 2

        def emit_sc_pair(u0, u1):
            # interleave the two heads' matmuls so their disjoint row groups
            # run concurrently on the PE array; nosync deps pin the queue
            # order (the scheduler otherwise sometimes groups by head)
            tiles = {}
            for u in (u0, u1):
                tiles[u] = stp.tile([128, 1024], F32, tag="st", name="stt")
                st_tiles[u] = tiles[u]
            prev = None
            for ci in range(2):
                for u in (u0, u1):
                    s, g, j = unit(u)
                    qh = qh_tiles[s]
                    lo = 64 * j
                    cc = 2 * g + ci
                    mm = nc.tensor.matmul(
                        tiles[u][:, ci * 512:(ci + 1) * 512],
                        khT[lo:lo + 64, cc * 128:(cc + 1) * 128],
                        qh[lo:lo + 64, :],
                        tile_position=(lo, 0))
                    if prev is not None:
                        bass._add_dep_helper(mm.ins, prev.ins, sync=False,
                                             reason="sc-interleave")
                    prev = mm

        def emit_act(u):
            ptt = ptp.tile([128, 1024], F16, tag="pt", name="pt")
            pt_tiles[u] = ptt
            stt = st_tiles.pop(u)
            # alternate exp between DVE (fast approx) and ACT (exact LUT),
            # keeping the sweep-boundary zone on ACT so the DVE is free for
            # the PV-accumulator drain there
            r = u % 32
            if r % 2 == 1 and 3 <= r <= 29:
                nc.vector.tensor_scalar(
                    ptt[:].bitcast(I16), stt[:], EXP_A, EXP_B,
                    mybir.AluOpType.mult, mybir.AluOpType.add)
            else:
                nc.scalar.activation(ptt[:], stt[:], Exp, scale=0.125)

        def emit_pv(u):
            s, g, j = unit(u)
            if g == 0:
                pvp_tiles[j] = pvpp.tile([128, 512], F32, tag="pvp", name="pvp")
            pvp = pvp_tiles[j]
            ptt = pt_tiles.pop(u)
            for ci in range(2):
                cc = 2 * g + ci
                nc.tensor.matmul(
                    pvp[0:65, :],
                    vh[:, cc * 130 + 65 * j: cc * 130 + 65 * j + 65],
                    ptt[:, ci * 512:(ci + 1) * 512],
                    start=(g == 0 and ci == 0), stop=(g == 15 and ci == 1))
            if g == 15:
                if j == 0:
                    cat_tiles[s] = catp.tile([128, 512], F16, tag="cat",
                                             name="cat")
                cat = cat_tiles[s]
                # extract the denominator row first (shortest path to the
                # recip->broadcast chain), then drain the accumulator; the
                # PSUM bank frees after both copies
                sums = normp.tile([1, 512], F32, tag="sums")
                nc.vector.tensor_copy(sums[0:1, :], pvp[64:65, :])
                pvc = normp.tile([64, 512], F32, tag="pvc")
                nc.vector.tensor_copy(pvc[:, :], pvp[0:64, :])
                rec = normp.tile([1, 512], F32, tag="rec")
                nc.vector.reciprocal_approx_fast(rec[0:1, :], sums[0:1, :])
                rep = normp.tile([64, 512], F32, tag="rep")
                nc.gpsimd.partition_broadcast(rep[:, :], rec[0:1, :])
                lo = 64 * j
                if s == NSW - 1:
                    # final sweep: per-column-block muls so the tail oproj
                    # can stream block-by-block
                    for i in range(4):
                        cs = slice(i * 128, (i + 1) * 128)
                        nc.vector.tensor_mul(cat[lo:lo + 64, cs],
                                             pvc[0:64, cs], rep[:, cs])
                else:
                    nc.vector.tensor_mul(cat[lo:lo + 64, :], pvc[0:64, :],
                                         rep[:])

        # ---- schedule ------------------------------------------------------
        # spread startup DMA issues across idle queues: sync+gpsimd carry the
        # kv stream, vector/scalar take the one-time loads
        prefetch_kv(0, nc.sync, split=nc.gpsimd)
        nc.scalar.dma_start(bqk[:, 0:1], bq.rearrange("(a p) -> p a", p=128))
        nc.scalar.dma_start(bqk[:, 1:2], bk.rearrange("(a p) -> p a", p=128))
        nc.scalar.dma_start(wv_sb[:].rearrange("p (m d) -> p m d", d=128),
                            wv.rearrange("(m p) d -> p m d", p=128))
        nc.scalar.dma_start(bv_sb[0:1, :], bv.rearrange("(a d) -> a d", a=1))
        nc.gpsimd.partition_broadcast(bv_rep[:, :], bv_sb[0:1, :])
        prefetch_q(0, nc.scalar)
        prefetch_kv(1, nc.gpsimd)
        prefetch_kv(2, nc.sync)
        nc.scalar.dma_start(wo_sb[:], wo)
        prefetch_q(1, nc.scalar)

        def hooks(u):
            s, r = u // 32, u % 32
            if u == 0:
                emit_qproj(0)
                emit_kproj(0)
                for ci in range(4):
                    emit_vproj(0, ci)
            # kv projection: group G over units 4(G-1)+1 .. 4(G-1)+4
            if 1 <= u <= 4 * (NGRP - 1):
                G, step = (u - 1) // 4 + 1, (u - 1) % 4
                if step == 0:
                    emit_kproj(G)
                emit_vproj(G, step)
                if step == 3 and G + 2 < NGRP:
                    prefetch_kv(G + 2, (nc.sync, nc.gpsimd)[G % 2])
            if r == 2 and s + 2 < NSW:
                prefetch_q(s + 2, (nc.gpsimd, nc.sync)[s % 2])
            if r == 16 and s + 1 < NSW:
                emit_qproj(s + 1)
            if s >= 1 and r in (14, 18, 22, 26):
                emit_oproj(s - 1, (r - 14) // 4)

        # pair-batched pipeline: PV(p-4,p-3) | ACT(p-2,p-1) | SC(p,p+1).
        # SC pairs are adjacent in the tensor queue so the two heads' row
        # groups run concurrently; PV runs of 4 expose only one LDWEIGHTS.
        for p in range(0, NU + 4, 2):
            u0, u1 = p, p + 1
            if u0 < NU:
                hooks(u0)
                hooks(u1)
            if u0 >= 4:
                emit_pv(u0 - 4)
                emit_pv(u1 - 4)
            if u0 >= 2 and u0 - 2 < NU:
                emit_act(u0 - 2)
                emit_act(u1 - 2)
            if u0 < NU:
                emit_sc_pair(u0, u1)

        for i in range(4):
            emit_oproj(NSW - 1, i)


_NC_CACHE = None


def _get_nc():
    global _NC_CACHE
    if _NC_CACHE is None:
        _NC_CACHE = _build_kernel()
    return _NC_CACHE


def kernel(q, k, v, Wq, bq, Wk, bk, Wv, bv, Wo, bo, trace=False):
    global LAST_RESULTS
    q = np.asarray(q, np.float32)
    k = np.asarray(k, np.float32)
    v = np.asarray(v, np.float32)

    qT16 = [np.ascontiguousarray(q[b].T).astype(np.float16) for b in range(2)]
    kT16 = [np.ascontiguousarray(k[b].T).astype(np.float16) for b in range(2)]
    vT16 = [np.ascontiguousarray(v[b].T).astype(np.float16) for b in range(2)]
    WqT = np.asarray(Wq, np.float32).T
    WkT = np.asarray(Wk, np.float32).T
    WvT = np.asarray(Wv, np.float32).T
    WoT = np.asarray(Wo, np.float32).T
    bq32 = np.asarray(bq, np.float32)
    bk32 = np.asarray(bk, np.float32)
    bv32 = np.asarray(bv, np.float32)

    in_maps = []
    for core in range(NCORES):
        b, hp = divmod(core, 4)
        sl = slice(128 * hp, 128 * (hp + 1))
        in_maps.append({
            "qT": qT16[b], "kT": kT16[b], "vT": vT16[b],
            "wq": np.ascontiguousarray(WqT[:, sl]).astype(np.float16),
            "wk": np.ascontiguousarray(WkT[:, sl]).astype(np.float16),
            "wv": np.ascontiguousarray(WvT[:, sl]).astype(np.float16),
            "wo": np.ascontiguousarray(WoT[sl, :]).astype(np.float16),
            "bq": np.ascontiguousarray(bq32[sl]),
            "bk": np.ascontiguousarray(bk32[sl]),
            "bv": np.ascontiguousarray(bv32[sl]),
        })

    nc = _get_nc()
    res = run_bass_kernel_spmd(nc, in_maps, core_ids=list(range(NCORES)),
                               trace=trace)
    LAST_RESULTS = res

    full = np.zeros((2, S, D), np.float32)
    for core in range(NCORES):
        b, hp = divmod(core, 4)
        full[b] += res.results[core]["out"].astype(np.float32)
    full += np.asarray(bo, np.float32)
    return full


# revision 31
# speedup vs baseline: 1.0477x; 1.0477x over previous
"""Multi-head attention (B=2, S=4096, D=512, H=8) on 8 TRN2 NeuronCores.

Sharding: core = (batch, head-pair). Each core projects q/k/v onto its two
heads' 128 dims over the full 4096-token sequence (no redundant work), runs
flash-style attention, and computes a partial output projection over its 128
cat dims. The host sums the 4 partial outputs per batch and adds bo.

exp() is split between the scalar engine (exact LUT) and the vector engine
(calibrated fp16 Schraudolph int-trick) to balance the two bottleneck
engines; OFFLOAD/16 of the score tiles take the vector path.
"""
import os
import sys

for _p in ("/opt/trn_rl_repo",):
    if _p not in sys.path:
        sys.path.insert(0, _p)

import numpy as np
from contextlib import ExitStack

import concourse.bass as bass
import concourse.bacc as bacc
import concourse.tile as tile
from concourse import mybir
from concourse.bass_utils import run_bass_kernel_spmd

F16 = mybir.dt.float16
F32 = mybir.dt.float32
I16 = mybir.dt.int16

D = 512          # d_model
DK = 64          # head dim
S = 4096         # sequence length
NCORES = 8
NSW = 8          # sweeps of 512 queries
NCH = 32         # 128-key chunks
NGRP = 8         # kv 512-key projection groups
NU = NSW * 32    # units: sweep x (16 granules x 2 heads)

# exp offload: units with u % 16 < OFFLOAD take the DVE fast-exp path
OFFLOAD = 7
LOG2E = 1.4426950408889634
EXP_A = float(0.125 * LOG2E * 1024.0)
EXP_B = float(15.0 * 1024.0 - 60.0)

LAST_RESULTS = None


def _build_kernel():
    nc = bacc.Bacc("TRN2", target_bir_lowering=False, debug=False,
                   num_devices=NCORES)

    qT = nc.dram_tensor("qT", [D, S], F16, kind="ExternalInput").ap()
    kT = nc.dram_tensor("kT", [D, S], F16, kind="ExternalInput").ap()
    vT = nc.dram_tensor("vT", [D, S], F16, kind="ExternalInput").ap()
    wq = nc.dram_tensor("wq", [D, 128], F16, kind="ExternalInput").ap()
    wk = nc.dram_tensor("wk", [D, 128], F16, kind="ExternalInput").ap()
    wv = nc.dram_tensor("wv", [D, 128], F16, kind="ExternalInput").ap()
    wo = nc.dram_tensor("wo", [128, D], F16, kind="ExternalInput").ap()
    bq = nc.dram_tensor("bq", [128], F32, kind="ExternalInput").ap()
    bk = nc.dram_tensor("bk", [128], F32, kind="ExternalInput").ap()
    bv = nc.dram_tensor("bv", [128], F32, kind="ExternalInput").ap()
    out = nc.dram_tensor("out", [S, D], F16, kind="ExternalOutput").ap()

    with tile.TileContext(nc) as tc:
        _emit(tc, qT, kT, vT, wq, wk, wv, wo, bq, bk, bv, out)

    nc.compile()
    return nc


def _emit(tc, qT, kT, vT, wq, wk, wv, wo, bq, bk, bv, out):
    nc = tc.nc
    Exp = mybir.ActivationFunctionType.Exp

    with ExitStack() as ctx:
        const = ctx.enter_context(tc.tile_pool(name="const", bufs=1))
        kvin = ctx.enter_context(tc.tile_pool(name="kvin", bufs=3))
        qinp = ctx.enter_context(tc.tile_pool(name="qinp", bufs=2))
        qhp = ctx.enter_context(tc.tile_pool(name="qhp", bufs=2))
        catp = ctx.enter_context(tc.tile_pool(name="catp", bufs=2))
        ptp = ctx.enter_context(tc.tile_pool(name="ptp", bufs=7))
        normp = ctx.enter_context(tc.tile_pool(name="normp", bufs=2))
        obp = ctx.enter_context(tc.tile_pool(name="obp", bufs=2))
        # PSUM: 3 x [128,1024] score tiles (6 banks) + 2 pv accumulators.
        # Projection tiles borrow "st" slots.
        stp = ctx.enter_context(tc.tile_pool(name="stp", bufs=3, space="PSUM"))
        pvpp = ctx.enter_context(tc.tile_pool(name="pvpp", bufs=2, space="PSUM"))
        pjp = stp

        # ---- persistent SBUF tensors -------------------------------------
        wq_sb = const.tile([128, 4 * 128], F16)   # [dm%128, m*128 + d2h]
        wk_sb = const.tile([128, 4 * 128], F16)
        wv_sb = const.tile([128, 4 * 128], F16)
        wo_sb = const.tile([128, D], F16)         # [cat dim, e]
        khT = const.tile([128, S], F16)           # [d2h, keys]
        vh = const.tile([128, NCH * 130], F16)    # per chunk: j*65+dd, col 64=1
        bqk = const.tile([128, 2], F32)           # col0 bq, col1 bk
        bv_sb = const.tile([1, 128], F32)
        bv_rep = const.tile([128, 128], F32)
        ones32 = const.tile([1, 128], F32)

        # ---- startup loads (critical-path first: wk -> kv group 0) -------
        nc.sync.dma_start(wk_sb[:].rearrange("p (m d) -> p m d", d=128),
                          wk.rearrange("(m p) d -> p m d", p=128))
        nc.gpsimd.dma_start(wq_sb[:].rearrange("p (m d) -> p m d", d=128),
                            wq.rearrange("(m p) d -> p m d", p=128))
        nc.vector.memset(ones32[:], 1.0)
        # ones column of every vh chunk (col 64 of each head block)
        vh_ones = vh[:, :].rearrange("p (a c) -> p a c", c=65)[:, :, 64:65]
        nc.vector.memset(vh_ones, 1.0)
        # preload the exp table set while startup DMAs run
        warm = normp.tile([1, 128], F16, tag="warm")
        nc.scalar.activation(warm[0:1, :], ones32[0:1, :], Exp)

        # ---- DMA: 512-key kv group / 512-query group (one start each) ----
        kv_tiles = {}

        def prefetch_kv(G, eng, split=None):
            kin = kvin.tile([128, 2048], F16, tag="kin", name="kin")
            vin = kvin.tile([128, 2048], F16, tag="vin", name="vin")
            kv_tiles[G] = (kin, vin)
            src_k = kT.rearrange("(m p) k -> p m k", p=128)[:, :, G * 512:(G + 1) * 512]
            src_v = vT.rearrange("(m p) k -> p m k", p=128)[:, :, G * 512:(G + 1) * 512]
            dst_k = kin[:].rearrange("p (m k) -> p m k", k=512)
            dst_v = vin[:].rearrange("p (m k) -> p m k", k=512)
            if split is None:
                split = nc.gpsimd if eng is nc.sync else nc.sync
            eng.dma_start(dst_k[:, 0:2], src_k[:, 0:2])
            split.dma_start(dst_k[:, 2:4], src_k[:, 2:4])
            eng.dma_start(dst_v[:, 0:2], src_v[:, 0:2])
            split.dma_start(dst_v[:, 2:4], src_v[:, 2:4])

        qin_tiles = {}

        def prefetch_q(s, eng):
            qin = qinp.tile([128, 2048], F16, tag="qin", name="qin")
            qin_tiles[s] = qin
            src = qT.rearrange("(m p) k -> p m k", p=128)[:, :, s * 512:(s + 1) * 512]
            eng.dma_start(qin[:].rearrange("p (m k) -> p m k", k=512), src)

        # ---- projections --------------------------------------------------
        qh_tiles = {}

        def emit_qproj(s):
            qin = qin_tiles.pop(s)
            qh = qhp.tile([128, 512], F16, tag="qh", name="qh")
            qh_tiles[s] = qh
            pj = pjp.tile([128, 512], F32, tag="st", name="pj")
            for m in range(4):
                nc.tensor.matmul(
                    pj[:],
                    wq_sb[:, m * 128:(m + 1) * 128],
                    qin[:, m * 512:(m + 1) * 512],
                    start=(m == 0), stop=(m == 3))
            nc.scalar.add(qh[:], pj[:], bqk[:, 0:1])

        def emit_kproj(G):
            kin, _ = kv_tiles[G]
            pj = pjp.tile([128, 512], F32, tag="st", name="pj")
            for m in range(4):
                nc.tensor.matmul(
                    pj[:],
                    wk_sb[:, m * 128:(m + 1) * 128],
                    kin[:, m * 512:(m + 1) * 512],
                    start=(m == 0), stop=(m == 3))
            nc.scalar.add(khT[:, G * 512:(G + 1) * 512], pj[:], bqk[:, 1:2])

        def emit_vproj(G, ci):
            _, vin = kv_tiles[G]
            cc = 4 * G + ci
            pj = pjp.tile([128, 128], F32, tag="st", name="pj")
            for m in range(4):
                nc.tensor.matmul(
                    pj[:],
                    vin[:, m * 512 + ci * 128: m * 512 + ci * 128 + 128],
                    wv_sb[:, m * 128:(m + 1) * 128],
                    start=(m == 0), stop=(m == 3))
            dst = vh[:, cc * 130:(cc + 1) * 130]
            dst = dst.rearrange("p (h c) -> p h c", c=65)[:, :, 0:64]
            nc.vector.tensor_add(
                dst,
                pj[:].rearrange("p (h c) -> p h c", c=64),
                bv_rep[:].rearrange("p (h c) -> p h c", c=64))

        # ---- output projection (partial: 128 cat dims) --------------------
        cat_tiles = {}
        ob_tiles = {}

        def emit_oproj(s, i):
            cat = cat_tiles[s]
            if i == 0:
                ob_tiles[s] = obp.tile([128, 2048], F16, tag="ob", name="ob")
            ob = ob_tiles[s]
            pj = pjp.tile([128, 512], F32, tag="st", name="pj")
            nc.tensor.matmul(pj[:], cat[:, i * 128:(i + 1) * 128], wo_sb[:])
            nc.scalar.copy(ob[:, i * 512:(i + 1) * 512], pj[:])
            if s == NSW - 1:
                # stream the tail out block by block
                nc.sync.dma_start(out[s * 512 + i * 128:s * 512 + (i + 1) * 128, :],
                                  ob[:, i * 512:(i + 1) * 512])
                if i == 3:
                    cat_tiles.pop(s)
                    ob_tiles.pop(s)
            elif i == 3:
                cat_tiles.pop(s)
                ob = ob_tiles.pop(s)
                dst = out[s * 512:(s + 1) * 512, :].rearrange(
                    "(i p) e -> p i e", p=128)
                nc.sync.dma_start(
                    dst, ob[:].rearrange("p (i e) -> p i e", e=512))

        # ---- attention pipeline -------------------------------------------
        # unit u: sweep s = u//32, g = (u%32)//2, head j = u%2
        # stages: SC at u, exp at u-2, PV at u-4
        pvp_tiles = {}
        st_tiles = {}
        pt_tiles = {}

        def unit(u):
            s, r = divmod(u, 32)
            return s, r // 2, r % 2

        def emit_sc_pair(u0, u1):
            # interleave the two heads' matmuls so their disjoint row groups
            # run concurrently on the PE array; nosync deps pin the queue
            # order (the scheduler otherwise sometimes groups by head)
            tiles = {}
            for u in (u0, u1):
                tiles[u] = stp.tile([128, 1024], F32, tag="st", name="stt")
                st_tiles[u] = tiles[u]
            prev = None
            for ci in range(2):
                for u in (u0, u1):
                    s, g, j = unit(u)
                    qh = qh_tiles[s]
                    lo = 64 * j
                    cc = 2 * g + ci
                    mm = nc.tensor.matmul(
                        tiles[u][:, ci * 512:(ci + 1) * 512],
                        khT[lo:lo + 64, cc * 128:(cc + 1) * 128],
                        qh[lo:lo + 64, :],
                        tile_position=(lo, 0))
                    if prev is not None:
                        bass._add_dep_helper(mm.ins, prev.ins, sync=False,
                                             reason="sc-interleave")
                    prev = mm

        def emit_act(u):
            ptt = ptp.tile([128, 1024], F16, tag="pt", name="pt")
            pt_tiles[u] = ptt
            stt = st_tiles.pop(u)
            # alternate exp between DVE (fast approx) and ACT (exact LUT),
            # keeping the sweep-boundary zone on ACT so the DVE is free for
            # the PV-accumulator drain there
            r = u % 32
            if r % 2 == 1 and 3 <= r <= 29:
                nc.vector.tensor_scalar(
                    ptt[:].bitcast(I16), stt[:], EXP_A, EXP_B,
                    mybir.AluOpType.mult, mybir.AluOpType.add)
            else:
                nc.scalar.activation(ptt[:], stt[:], Exp, scale=0.125)

        def emit_pv(u):
            s, g, j = unit(u)
            if g == 0:
                pvp_tiles[j] = pvpp.tile([128, 512], F32, tag="pvp", name="pvp")
            pvp = pvp_tiles[j]
            ptt = pt_tiles.pop(u)
            for ci in range(2):
                cc = 2 * g + ci
                nc.tensor.matmul(
                    pvp[0:65, :],
                    vh[:, cc * 130 + 65 * j: cc * 130 + 65 * j + 65],
                    ptt[:, ci * 512:(ci + 1) * 512],
                    start=(g == 0 and ci == 0), stop=(g == 15 and ci == 1))
            if g == 15:
                if j == 0:
                    cat_tiles[s] = catp.tile([128, 512], F16, tag="cat",
                                             name="cat")
                cat = cat_tiles[s]
                # single copy frees the PSUM bank for the next sweep's PV;
                # the rest of the normalize chain trails off-critical-path
                pvc = normp.tile([65, 512], F32, tag="pvc")
                nc.vector.tensor_copy(pvc[:, :], pvp[0:65, :])
                sums = normp.tile([1, 512], F32, tag="sums")
                nc.vector.tensor_copy(sums[0:1, :], pvc[64:65, :])
                rec = normp.tile([1, 512], F32, tag="rec")
                nc.vector.reciprocal_approx_fast(rec[0:1, :], sums[0:1, :])
                rep = normp.tile([64, 512], F32, tag="rep")
                nc.gpsimd.partition_broadcast(rep[:, :], rec[0:1, :])
                lo = 64 * j
                if s == NSW - 1:
                    # final sweep: per-column-block muls so the tail oproj
                    # can stream block-by-block
                    for i in range(4):
                        cs = slice(i * 128, (i + 1) * 128)
                        nc.vector.tensor_mul(cat[lo:lo + 64, cs],
                                             pvc[0:64, cs], rep[:, cs])
                else:
                    nc.vector.tensor_mul(cat[lo:lo + 64, :], pvc[0:64, :],
                                         rep[:])

        # ---- schedule ------------------------------------------------------
        # spread startup DMA issues across idle queues: sync+gpsimd carry the
        # kv stream, vector/scalar take the one-time loads
        prefetch_kv(0, nc.sync, split=nc.gpsimd)
        nc.sync.dma_start(bqk[:, 0:1], bq.rearrange("(a p) -> p a", p=128))
        nc.sync.dma_start(bqk[:, 1:2], bk.rearrange("(a p) -> p a", p=128))
        nc.gpsimd.dma_start(wv_sb[:].rearrange("p (m d) -> p m d", d=128),
                            wv.rearrange("(m p) d -> p m d", p=128))
        nc.gpsimd.dma_start(bv_sb[0:1, :], bv.rearrange("(a d) -> a d", a=1))
        nc.gpsimd.partition_broadcast(bv_rep[:, :], bv_sb[0:1, :])
        prefetch_q(0, nc.sync)
        prefetch_kv(1, nc.gpsimd)
        prefetch_kv(2, nc.sync)
        nc.gpsimd.dma_start(wo_sb[:], wo)
        prefetch_q(1, nc.gpsimd)

        def hooks(u):
            s, r = u // 32, u % 32
            if u == 0:
                emit_qproj(0)
                emit_kproj(0)
                for ci in range(4):
                    emit_vproj(0, ci)
            # kv projection: group G over units 4(G-1)+1 .. 4(G-1)+4
            if 1 <= u <= 4 * (NGRP - 1):
                G, step = (u - 1) // 4 + 1, (u - 1) % 4
                if step == 0:
                    emit_kproj(G)
                emit_vproj(G, step)
                if step == 3 and G + 2 < NGRP:
                    prefetch_kv(G + 2, (nc.sync, nc.gpsimd)[G % 2])
            if r == 2 and s + 2 < NSW:
                prefetch_q(s + 2, (nc.gpsimd, nc.sync)[s % 2])
            if r == 16 and s + 1 < NSW:
                emit_qproj(s + 1)
            if s >= 1 and r in (14, 18, 22, 26):
                emit_oproj(s - 1, (r - 14) // 4)

        # pair-batched pipeline: PV(p-4,p-3) | ACT(p-2,p-1) | SC(p,p+1).
        # SC pairs are adjacent in the tensor queue so the two heads' row
        # groups run concurrently; PV runs of 4 expose only one LDWEIGHTS.
        for p in range(0, NU + 4, 2):
            u0, u1 = p, p + 1
            if u0 < NU:
                hooks(u0)
                hooks(u1)
            if u0 >= 4:
                emit_pv(u0 - 4)
                emit_pv(u1 - 4)
            if u0 >= 2 and u0 - 2 < NU:
                emit_act(u0 - 2)
                emit_act(u1 - 2)
            if u0 < NU:
                emit_sc_pair(u0, u1)

        for i in range(4):
            emit_oproj(NSW - 1, i)


_NC_CACHE = None


def _get_nc():
    global _NC_CACHE
    if _NC_CACHE is None:
        _NC_CACHE = _build_kernel()
    return _NC_CACHE


def kernel(q, k, v, Wq, bq, Wk, bk, Wv, bv, Wo, bo, trace=False):
    global LAST_RESULTS
    q = np.asarray(q, np.float32)
    k = np.asarray(k, np.float32)
    v = np.asarray(v, np.float32)

    qT16 = [np.ascontiguousarray(q[b].T).astype(np.float16) for b in range(2)]
    kT16 = [np.ascontiguousarray(k[b].T).astype(np.float16) for b in range(2)]
    vT16 = [np.ascontiguousarray(v[b].T).astype(np.float16) for b in range(2)]
    WqT = np.asarray(Wq, np.float32).T
    WkT = np.asarray(Wk, np.float32).T
    WvT = np.asarray(Wv, np.float32).T
    WoT = np.asarray(Wo, np.float32).T
    bq32 = np.asarray(bq, np.float32)
    bk32 = np.asarray(bk, np.float32)
    bv32 = np.asarray(bv, np.float32)

    in_maps = []
    for core in range(NCORES):
        b, hp = divmod(core, 4)
        sl = slice(128 * hp, 128 * (hp + 1))
        in_maps.append({
            "qT": qT16[b], "kT": kT16[b], "vT": vT16[b],
            "wq": np.ascontiguousarray(WqT[:, sl]).astype(np.float16),
            "wk": np.ascontiguousarray(WkT[:, sl]).astype(np.float16),
            "wv": np.ascontiguousarray(WvT[:, sl]).astype(np.float16),
            "wo": np.ascontiguousarray(WoT[sl, :]).astype(np.float16),
            "bq": np.ascontiguousarray(bq32[sl]),
            "bk": np.ascontiguousarray(bk32[sl]),
            "bv": np.ascontiguousarray(bv32[sl]),
        })

    nc = _get_nc()
    res = run_bass_kernel_spmd(nc, in_maps, core_ids=list(range(NCORES)),
                               trace=trace)
    LAST_RESULTS = res

    full = np.zeros((2, S, D), np.float32)
    for core in range(NCORES):
        b, hp = divmod(core, 4)
        full[b] += res.results[core]["out"].astype(np.float32)
    full += np.asarray(bo, np.float32)
    return full
